# revision 1
# baseline (speedup 1.0000x reference)
"""GCLSTM (Chebyshev graph-conv LSTM cell) on 8 Trainium2 NeuronCores.

Strategy (per sharding hint): dense Laplacian row-sharded across 8 cores.
Each core owns a 1024-row block of L, stored transposed (L^T columns) in
DRAM as bf16 and streamed through SBUF. The Chebyshev recurrence
T_{k+1} = 2 L T_k - T_{k-1} runs on the TensorEngine in *transposed*
orientation: VT = (L_m @ U)^T = matmul(lhsT=U_chunk, rhs=LT_chunk), which
directly yields the [feature, node] layout the gate matmuls consume.
Between steps, each core's new T_k block is PE-transposed back to
node-major and AllGathered in four node-quarter chunks, so the next
step's contraction stream (which walks global node chunks in quarter
order) starts as soon as the first quarter lands. Gate contributions
sum_k T_k W_k^T accumulate per-k into SBUF f32 right after each
recurrence step, filling the AllGather wait windows. The LSTM epilogue
runs in transposed layout, pipelined across ACT/Pool/DVE in half-tiles.
"""

import os
import sys
from contextlib import ExitStack

import numpy as np

for _p in ("/opt/trn_rl_repo", "/root/.axon_site/_ro/trn_rl_repo"):
    if os.path.isdir(_p) and _p not in sys.path:
        sys.path.insert(0, _p)

import ml_dtypes  # noqa: E402

from concourse import bacc, mybir, tile  # noqa: E402
from concourse.bass_utils import run_bass_kernel_spmd  # noqa: E402
from concourse.kernels.tile_matmul import make_identity  # noqa: E402

BF16 = mybir.dt.bfloat16
F32 = mybir.dt.float32
AF = mybir.ActivationFunctionType
bf16 = ml_dtypes.bfloat16

N = 8192          # nodes
C = 8             # cores
NL = N // C       # rows per core (1024)
K = 4             # Chebyshev order
H = 256           # hidden
I_ = 128          # input feature size
F = H + I_        # fused recurrence width (384)
FT = F // 128     # feature tiles (3)
NT = N // 128     # global node tiles (64)
MT = H // 128     # output h' tiles (2)
NHALF = 2         # 512-wide halves of the local node dim
NQ = 4            # node quarters for AllGather chunking


def _ci_of(q, r, j):
    return r * 8 + q * 2 + j


def _build_nc(reps=1, pad_cycles=0, collectives=True):
    nc = bacc.Bacc("TRN2", target_bir_lowering=False, debug=False,
                   num_devices=C if collectives else 1)

    lt = nc.dram_tensor("lt", [N, NL], BF16, kind="ExternalInput")
    u0 = nc.dram_tensor("u0", [N, F], BF16, kind="ExternalInput")
    u0t = nc.dram_tensor("u0t", [F, NL], F32, kind="ExternalInput")
    w = nc.dram_tensor("w", [128, 4 * K * FT * MT * 128], BF16,
                       kind="ExternalInput")
    bz = nc.dram_tensor("bz", [4, MT, 128, NL], F32, kind="ExternalInput")
    ctin = nc.dram_tensor("ctin", [MT, 128, NL], F32, kind="ExternalInput")
    hout = nc.dram_tensor("hout", [MT, 128, NL], F32, kind="ExternalOutput")
    cout = nc.dram_tensor("cout", [MT, 128, NL], F32, kind="ExternalOutput")

    with tile.TileContext(nc) as tc, ExitStack() as ctx:
        const = ctx.enter_context(tc.tile_pool(name="const", bufs=1))
        ident = const.tile([128, 128], BF16, name="ident")
        make_identity(nc, ident)

        wpool = ctx.enter_context(tc.tile_pool(name="wpool", bufs=6))
        wtiles = {}

        persist = ctx.enter_context(tc.tile_pool(name="persist", bufs=1))
        u_pool = ctx.enter_context(tc.tile_pool(name="u_pool", bufs=2))
        lt_pool = ctx.enter_context(tc.tile_pool(name="lt_pool", bufs=6))
        scratch = ctx.enter_context(tc.tile_pool(name="scratch", bufs=3))
        mtmp_p = ctx.enter_context(tc.tile_pool(name="mtmp_p", bufs=4))
        ztmp_p = ctx.enter_context(tc.tile_pool(name="ztmp_p", bufs=2))
        b_pool = ctx.enter_context(tc.tile_pool(name="b_pool", bufs=2))
        v_pool = ctx.enter_context(tc.tile_pool(name="v_pool", bufs=4))
        epi = ctx.enter_context(tc.tile_pool(name="epi", bufs=2))
        vt_pool = ctx.enter_context(
            tc.tile_pool(name="vt_pool", bufs=1, space="PSUM"))
        zp_pool = ctx.enter_context(
            tc.tile_pool(name="zp_pool", bufs=2, space="PSUM"))
        dram = ctx.enter_context(tc.tile_pool(name="dram", bufs=1,
                                              space="DRAM"))

        def w_load(g, s):
            t = wpool.tile([128, FT * MT * 128], BF16, tag="wt",
                           name=f"wt_{g}_{s}")
            c0 = ((g * K + s) * FT) * MT * 128
            c1 = ((g * K + s + 1) * FT) * MT * 128
            nc.sync.dma_start(t[:], w[:, c0:c1])
            return t

        def w_slice(g, s, ft, mt):
            idx = (ft * MT + mt) * 128
            return wtiles[g, s][:, idx:idx + 128]

        for rep in range(reps):
            a1 = [persist.tile([128, NL], F32, tag=f"a1_{ft}",
                               name=f"a1_{ft}") for ft in range(FT)]
            zacc = [[persist.tile([128, NL], F32, tag=f"z_{g}_{mt}",
                                  name=f"z_{g}_{mt}")
                     for mt in range(MT)] for g in range(4)]

            ag_in = {}
            ag_out = {}
            for s in (1, 2):
                for q in range(NQ):
                    ag_in[s, q] = dram.tile([2 * 128, F], BF16,
                                            name=f"ag_in_{s}_{q}")
                    ag_out[s, q] = dram.tile([C * 2 * 128, F], BF16,
                                             addr_space="Shared",
                                             name=f"ag_out_{s}_{q}")

            def gate_round(s, b_tiles):
                """Accumulate sum_f W_g[s]^T-slices @ T_s^T into zacc.
                Round 0 initializes zacc = W-term + bias (bias folded in
                up front so the epilogue reads zacc directly)."""
                for g in range(4):
                    wtiles[g, s] = w_load(g, s)
                for g in range(4):
                    for mt in range(MT):
                        for h in range(NHALF):
                            sl = slice(h * 512, (h + 1) * 512)
                            zp = zp_pool.tile([128, 512], F32, tag="zp",
                                              name=f"zp_{s}_{g}_{mt}_{h}")
                            for ft in range(FT):
                                nc.tensor.matmul(
                                    zp[:], w_slice(g, s, ft, mt),
                                    b_tiles[ft][:, sl],
                                    start=(ft == 0), stop=(ft == FT - 1))
                            dst = zacc[g][mt][:, sl]
                            if s == 0:
                                nc.vector.tensor_add(
                                    dst, bias_t[g, mt][:, sl], zp[:])
                            else:
                                nc.vector.tensor_add(dst, dst, zp[:])

            # quarter-ordered view of u0 (ci = r*8 + q*2 + j)
            u0_q = u0[:].rearrange("(r q j p) f -> p q j r f", r=8, q=NQ,
                                   j=2, p=128)

            bias_t = {}

            def load_biases():
                for g in range(4):
                    for mt in range(MT):
                        t = epi.tile([128, NL], F32, tag="bzt",
                                     name=f"bzt_{g}_{mt}", bufs=8)
                        nc.sync.dma_start(t[:], bz[g, mt])
                        bias_t[g, mt] = t

            # ---- round 0 inputs: T_0^T local block from host
            a0_tiles = []
            b0_tiles = []
            for ft in range(FT):
                st = scratch.tile([128, NL], F32, tag="scratch",
                                  name=f"u0t_{ft}")
                nc.sync.dma_start(st[:], u0t[ft * 128:(ft + 1) * 128, :])
                b0 = b_pool.tile([128, NL], BF16, tag=f"b{ft}",
                                 name=f"b0_{ft}")
                nc.gpsimd.tensor_copy(b0[:], st[:])
                a0_tiles.append(st)
                b0_tiles.append(b0)

            prev_b = b0_tiles
            # ---- recurrence steps s = 1..3 producing T_s
            for s in (1, 2, 3):
                vt = [[vt_pool.tile([128, 512], F32, tag=f"vt{ft}_{h}",
                                    name=f"vt_{s}_{ft}_{h}")
                       for h in range(NHALF)] for ft in range(FT)]
                first = True
                for q in range(NQ):
                    # U_{s-1} quarter fill (node-major bf16, rolling buffer)
                    uq = u_pool.tile([128, 16 * F], BF16, tag="uq",
                                     name=f"uq_{s}_{q}")
                    uq_v = uq[:].rearrange("p (a f) -> p a f", f=F)
                    uq_rj = uq[:].rearrange("p (r j f) -> p j r f", r=8, j=2)
                    if s == 1:
                        for j in range(2):
                            if q == 0:
                                for rh in range(2):
                                    nc.sync.dma_start(
                                        uq_rj[:, j, rh * 4:(rh + 1) * 4],
                                        u0_q[:, q, j, rh * 4:(rh + 1) * 4])
                            else:
                                nc.sync.dma_start(uq_rj[:, j], u0_q[:, q, j])
                    else:
                        src = ag_out[s - 1, q][:].rearrange(
                            "(a p) f -> p a f", p=128)
                        nc.sync.dma_start(uq_v, src)
                    for r in range(8):
                        ci0 = _ci_of(q, r, 0)
                        ltt = lt_pool.tile([128, 2 * NL], BF16, tag="lt",
                                           name=f"lt_{s}_{q}_{r}")
                        nc.sync.dma_start(
                            ltt[:].rearrange("p (a r) -> p a r", r=NL),
                            lt[:].rearrange("(a p) r -> p a r", p=128)
                            [:, ci0:ci0 + 2, :])
                        for j in range(2):
                            li = r * 2 + j
                            for ft in range(FT):
                                lhsT = uq[:, li * F + ft * 128:
                                          li * F + (ft + 1) * 128]
                                for h in range(NHALF):
                                    nc.tensor.matmul(
                                        vt[ft][h][:], lhsT,
                                        ltt[:, j * NL + h * 512:
                                            j * NL + (h + 1) * 512],
                                        start=first, stop=(q == NQ - 1
                                                           and r == 7
                                                           and j == 1))
                            first = False

                # evict + Chebyshev combine (per ft, per half)
                b_tiles = []
                for ft in range(FT):
                    b_t = b_pool.tile([128, NL], BF16, tag=f"b{ft}",
                                      name=f"b{s}_{ft}")
                    for h in range(NHALF):
                        sl = slice(h * 512, (h + 1) * 512)
                        if s == 1:
                            nc.scalar.activation(a1[ft][:, sl],
                                                 vt[ft][h][:], AF.Copy)
                            nc.vector.tensor_copy(b_t[:, sl], a1[ft][:, sl])
                        else:
                            prev = a0_tiles[ft] if s == 2 else a1[ft]
                            tmp = mtmp_p.tile([128, 512], F32, tag="mtmp",
                                              name=f"t2v_{s}_{ft}_{h}")
                            nc.scalar.activation(tmp[:], vt[ft][h][:],
                                                 AF.Copy, scale=2.0)
                            nc.vector.tensor_sub(b_t[:, sl], tmp[:],
                                                 prev[:, sl])
                    b_tiles.append(b_t)

                # transpose to node-major + AllGather per node quarter
                if s < 3:
                    for q in range(NQ):
                        vq = v_pool.tile([128, 2 * F], BF16, tag="vsb",
                                         name=f"vq_{s}_{q}")
                        for ft in range(FT):
                            for ntl in range(2):
                                nt_ = 2 * q + ntl
                                tp = zp_pool.tile(
                                    [128, 128], BF16, tag="zp",
                                    name=f"tp_{s}_{q}_{ft}_{ntl}")
                                nc.tensor.transpose(
                                    tp[:],
                                    b_tiles[ft][:, nt_ * 128:(nt_ + 1) * 128],
                                    ident[:])
                                nc.vector.tensor_copy(
                                    vq[:, (ntl * FT + ft) * 128:
                                       (ntl * FT + ft + 1) * 128],
                                    tp[:])
                        nc.sync.dma_start(
                            ag_in[s, q][:].rearrange(
                                "(ntl p) (ft f) -> p ntl ft f",
                                p=128, f=128),
                            vq[:].rearrange("p (ntl ft f) -> p ntl ft f",
                                            ft=FT, f=128))
                        if collectives:
                            nc.gpsimd.collective_compute(
                                "AllGather", mybir.AluOpType.bypass,
                                replica_groups=[list(range(C))],
                                ins=[ag_in[s, q].opt()],
                                outs=[ag_out[s, q].opt()])
                        else:
                            nc.sync.dma_start(ag_out[s, q][0:2 * 128, :],
                                              ag_in[s, q][:])

                if s == 1:
                    load_biases()
                gate_round(s - 1, prev_b)
                prev_b = b_tiles

            gate_round(3, prev_b)

            # ---- epilogue: LSTM cell update in transposed half-tiles
            for mt in range(MT):
                for h in range(NHALF):
                    sl = slice(h * 512, (h + 1) * 512)
                    acts = []
                    for g in range(4):
                        av = ztmp_p.tile([128, 512], F32, tag="atmp",
                                         name=f"act_{g}_{mt}_{h}", bufs=6)
                        nc.scalar.activation(av[:], zacc[g][mt][:, sl],
                                             AF.Tanh if g == 3
                                             else AF.Sigmoid)
                        acts.append(av)
                    f_t, i_t, o_t, ch_t = acts
                    cot = epi.tile([128, 512], F32, tag="cot",
                                   name=f"cot_{mt}_{h}", bufs=2)
                    nc.scalar.dma_start(cot[:], ctin[mt][:, sl])
                    p1 = ztmp_p.tile([128, 512], F32, tag="ptmp",
                                     name=f"p1_{mt}_{h}")
                    nc.vector.tensor_mul(p1[:], i_t[:], ch_t[:])
                    p2 = ztmp_p.tile([128, 512], F32, tag="ptmp2",
                                     name=f"p2_{mt}_{h}")
                    nc.gpsimd.tensor_mul(p2[:], f_t[:], cot[:])
                    cn = ztmp_p.tile([128, 512], F32, tag="cnh",
                                     name=f"cn_{mt}_{h}")
                    nc.vector.tensor_add(cn[:], p1[:], p2[:])
                    nc.gpsimd.dma_start(cout[mt][:, sl], cn[:])
                    th = ztmp_p.tile([128, 512], F32, tag="ttmp",
                                     name=f"th_{mt}_{h}")
                    nc.scalar.activation(th[:], cn[:], AF.Tanh)
                    hn = ztmp_p.tile([128, 512], F32, tag="hnh",
                                     name=f"hn_{mt}_{h}")
                    nc.gpsimd.tensor_mul(hn[:], o_t[:], th[:])
                    nc.scalar.dma_start(hout[mt][:, sl], hn[:])

        if pad_cycles:
            _emit_pad(nc, pad_cycles)

    nc.compile()
    return nc


def _emit_pad(nc, pad_cycles):
    CH = 60000
    n = int(pad_cycles) // CH
    for _ in range(n):
        nc.scalar.nop(cycle_cnt=CH, nofuse=True)


_NC_CACHE = {}


def _get_nc(reps=1, pad_cycles=0, collectives=True):
    key = (reps, pad_cycles, collectives)
    if key not in _NC_CACHE:
        _NC_CACHE[key] = _build_nc(reps, pad_cycles, collectives)
    return _NC_CACHE[key]


def prepare_in_maps(x, h, c, lap_vals, pos, W_fh, W_fx, bf, W_ih, W_ix, bi,
                    W_oh, W_ox, bo, W_ch, W_cx, bc):
    x = np.asarray(x, np.float32)
    h = np.asarray(h, np.float32)
    c = np.asarray(c, np.float32)
    lap_vals = np.asarray(lap_vals, np.float32)
    pos = np.asarray(pos)

    L = np.zeros((N, N), np.float32)
    L[pos[:, 0], pos[:, 1]] = lap_vals  # last write wins

    U0 = np.concatenate([h, x], axis=1)           # [N, 384]
    u0_b = np.ascontiguousarray(U0).astype(bf16)

    # gate weights -> SBUF-layout lhsT tiles [128, (g k ft mt) * 128]
    whs = [W_fh, W_ih, W_oh, W_ch]
    wxs = [W_fx, W_ix, W_ox, W_cx]
    wfull = np.zeros((128, 4, K, FT, MT, 128), np.float32)
    for g in range(4):
        wh = np.asarray(whs[g], np.float32)
        wx = np.asarray(wxs[g], np.float32)
        for s in range(K):
            wgt = np.concatenate([wh[s], wx[s]], axis=1).T  # [384,256] (f,h')
            # [ft, p, mt, f2] -> [p, ft, mt, f2]
            wfull[:, g, s] = wgt.reshape(FT, 128, MT, 128).transpose(1, 0, 2, 3)
    w_host = np.ascontiguousarray(
        wfull.reshape(128, 4 * K * FT * MT * 128)).astype(bf16)

    bzs = [np.asarray(b, np.float32) for b in (bf, bi, bo, bc)]

    in_maps = []
    for ci in range(C):
        r0, r1 = ci * NL, (ci + 1) * NL
        lt_c = np.ascontiguousarray(L[r0:r1, :].T).astype(bf16)
        u0t_c = np.ascontiguousarray(U0[r0:r1, :].T)
        bz_c = np.stack([b[r0:r1].T.reshape(MT, 128, NL) for b in bzs])
        ct_c = np.ascontiguousarray(c[r0:r1].T).reshape(MT, 128, NL)
        in_maps.append({
            "lt": lt_c,
            "u0": u0_b,
            "u0t": u0t_c,
            "w": w_host,
            "bz": bz_c,
            "ctin": np.ascontiguousarray(ct_c),
        })
    return in_maps


def assemble_outputs(results):
    h_new = np.empty((N, H), np.float32)
    c_new = np.empty((N, H), np.float32)
    for ci in range(C):
        r0, r1 = ci * NL, (ci + 1) * NL
        h_new[r0:r1] = results[ci]["hout"].reshape(H, NL).T
        c_new[r0:r1] = results[ci]["cout"].reshape(H, NL).T
    return h_new, c_new


def kernel(**inputs):
    nc = _get_nc()
    in_maps = prepare_in_maps(**inputs)
    res = run_bass_kernel_spmd(nc, in_maps, list(range(C)))
    return assemble_outputs(res.results)

